# revision 13
# baseline (speedup 1.0000x reference)
"""Expert-parallel grouped MLP (MoE routing) for Trainium2.

Problem: x[16384,1024] fp32, w1[8,1024,4096], w2[8,4096,1024],
rows_per_expert=2048.  out = gelu(x_e @ w1[e]) @ w2[e] per expert group.

Sharding: one expert per NeuronCore (E=8 == n_cores).  Each core runs an
identical Bass program on its own expert's slice; no collectives.  The host
pre-permutes each operand so every DMA chunk is a fully contiguous DRAM
region with 1-2KB per-partition lines:
    x  -> [NBLK, 128, HO, T_BLK]   (xp[b,p,h,ti]  = x[b*T_BLK+ti, h*128+p])
    w1 -> [NG*HO, 128, 1024]       chunk c=g*8+h: w1g[c,p,fl*128+fi]
                                     = w1[h*128+p, (g*8+fl)*128+fi]
    w2 -> [HO, 128, F]             (w2p[h,p,f*128+hi] = w2[f*128+p, h*128+hi])
    out <- [NBLK, HO, 128, T_BLK]  (out4[b,h,p,ti] = out[b*T_BLK+ti, h*128+p])
Activations stay in [feature, token] orientation through both GEMMs:
    GEMM1: interT[f,t] = sum_h w1[h,f] * xT[h,t]    (lhsT = w1 tile)
    gelu on PSUM -> SBUF (bf16)
    GEMM2: outT[h,t]  = sum_f w2[f,h] * interT[f,t]  (lhsT = w2 tile)
Matmuls run in bf16 (fp32 PSUM accumulate) - fp32 matmul is 4x slower on
the PE array.  Weights are SBUF-resident (64KB/partition each); tokens are
processed in 4 blocks of 512 so interT fits in SBUF.

Startup: per-engine HWDGE triggers land on one DMA queue each (SP and
Activation), each queue runs its copies serially at ~130-260 B/ns and
active queues share ~330 B/ns, so the startup operands are interleaved
across both queues in need-order.  Block 0's first f-group runs GEMM1
h-OUTER across all 8 PSUM banks (f-tiles 0..7 accumulate in parallel), so
each 1.73us h-step consumes only 384KB (one x chunk + one w1 chunk) instead
of the whole 1.25MB gating the first f-chain.  N=128 warm-up matmuls on a
zeroed tile ramp the PE clock (HAM) while the first chunks stream in.
Remaining w1/w2 chunks are gated on compute progress via the SP queue.
"""

import numpy as np
import ml_dtypes

E = 8
H = 1024
F = 4096
T_PER_E = 2048
T_BLK = 512
NBLK = T_PER_E // T_BLK
P = 128
HO = H // P    # 8 contraction chunks for GEMM1
FO = F // P    # 32 f-tiles
NG = 4         # f-groups of 8 tiles (one w1 chunk per (group, h))
NW2 = 8        # w2 staged in HO chunks
NWARM = 32     # PE warm-up matmuls (N=128: cover until startup DMAs land)

TRACE = False          # test.py sets kernel.TRACE = True for profiling
LAST_RESULTS = None    # BassKernelResults of the most recent run

_nc_cache = None


def _build_nc():
    import concourse.mybir as mybir
    import concourse.tile as tile
    from concourse import bacc
    from concourse.tile_rust import add_dep_helper

    bf16 = mybir.dt.bfloat16
    f32 = mybir.dt.float32
    GELU = mybir.ActivationFunctionType.Gelu_apprx_tanh

    nc = bacc.Bacc("TRN2", target_bir_lowering=False, debug=False)

    xp = nc.dram_tensor("xp", [NBLK, P, HO, T_BLK], bf16, kind="ExternalInput").ap()
    w1p = nc.dram_tensor("w1p", [NG * HO, P, 1024], bf16, kind="ExternalInput").ap()
    w2p = nc.dram_tensor("w2p", [HO, P, F], bf16, kind="ExternalInput").ap()
    # Output in bf16: halves the store traffic draining at the kernel tail;
    # the host upcasts to fp32.  The added rounding (~1e-3 relative, on top
    # of the ~3.4e-3 from the bf16 matmuls) is negligible.
    out4 = nc.dram_tensor("out4", [NBLK, HO, P, T_BLK], bf16, kind="ExternalOutput").ap()

    with tile.TileContext(nc) as tc:
        with (
            tc.tile_pool(name="wpool", bufs=1) as wpool,
            tc.tile_pool(name="xpool", bufs=2) as xpool,
            tc.tile_pool(name="ipool", bufs=1) as ipool,
            tc.tile_pool(name="opool", bufs=3) as opool,
            tc.tile_pool(name="psp", bufs=8, space="PSUM") as psp,
        ):
            # PE warm-up: dummy N=128 matmuls on a small zeroed tile keep the
            # PE busy (and its clock ramping) until the first real operands
            # are in SBUF.
            warm = wpool.tile([P, P], bf16, tag="warm")
            nc.any.memset(warm[:], 0.0)
            for _ in range(NWARM):
                wp = psp.tile([P, T_BLK], f32, tag="pst")
                nc.tensor.matmul(wp[:, 0:P], warm[:], warm[:], start=True, stop=True)

            # w1 SBUF layout [P, NG*HO, 1024]: chunk c = g*8+h holds f-tiles
            # g*8..g*8+7 for contraction row-block h.
            # lhsT for (f, h) = w1_sb[:, (f//8)*8+h, (f%8)*128:(f%8+1)*128]
            # w2 layout [P, HO, F]: lhsT for (f, h) = w2_sb[:, h, f*128:(f+1)*128]
            w1_sb = wpool.tile([P, NG * HO, 1024], bf16, tag="w1sb")
            w2_sb = wpool.tile([P, HO, F], bf16, tag="w2sb")

            def w1_lhsT(f, h):
                fl = f % 8
                return w1_sb[:, (f // 8) * 8 + h, fl * P:(fl + 1) * P]

            # Startup triggers, interleaved across the SP and Activation
            # queues in need-order for block 0's h-outer group 0:
            #   step h consumes x chunk h and w1 chunk (0, h).
            # SP's queue carries the whole w1 stream in chunk order (256KB
            # per 1.73us h-step, under its ~165 B/ns solo rate); Scalar's
            # queue carries all of xb0 (128KB per step).  Each queue's
            # cumulative-need curve then has >= 0.5us of slack per step.
            xb0 = xpool.tile([P, HO, T_BLK], bf16, tag="xb")
            w1_dmas = [None] * (NG * HO)
            w1_dmas[0] = nc.sync.dma_start(w1_sb[:, 0, :], w1p[0])
            nc.scalar.dma_start(xb0[:, 0, :], xp[0, :, 0, :])
            nc.scalar.dma_start(xb0[:, 1, :], xp[0, :, 1, :])
            nc.scalar.dma_start(xb0[:, 2:4, :], xp[0, :, 2:4, :])
            nc.scalar.dma_start(xb0[:, 4:6, :], xp[0, :, 4:6, :])
            nc.scalar.dma_start(xb0[:, 6:8, :], xp[0, :, 6:8, :])
            for c in range(1, NG * HO):
                w1_dmas[c] = nc.sync.dma_start(w1_sb[:, c, :], w1p[c])
            w2_dmas = [
                nc.sync.dma_start(w2_sb[:, h, :], w2p[h]) for h in range(NW2)
            ]

            anchor = {}  # chunk c -> first matmul instruction reading it
            HB = T_BLK // 2

            for b in range(NBLK):
                if b == 0:
                    xb = xb0
                else:
                    xb = xpool.tile([P, HO, T_BLK], bf16, tag="xb")
                    nc.sync.dma_start(xb[:], xp[b])

                it = ipool.tile([P, FO, T_BLK], bf16, tag="inter")
                for g in range(NG):
                    if b == 0 and g == 0:
                        # h-outer: all 8 PSUM banks accumulate f-tiles 0..7
                        # in parallel; each h-step needs only chunk (0,h).
                        pss = [psp.tile([P, T_BLK], f32, tag="pst",
                                        name=f"ps_g0_{fl}")
                               for fl in range(8)]
                        for h in range(HO):
                            for fl in range(8):
                                mm = nc.tensor.matmul(
                                    pss[fl][:],
                                    w1_lhsT(g * 8 + fl, h),
                                    xb[:, h, :],
                                    start=(h == 0),
                                    stop=(h == HO - 1),
                                )
                                if fl == 0:
                                    anchor[h] = mm
                        for fl in range(8):
                            nc.scalar.activation(it[:, fl, :], pss[fl][:], GELU)
                    else:
                        for fl in range(8):
                            f = g * 8 + fl
                            ps = psp.tile([P, T_BLK], f32, tag="pst")
                            for h in range(HO):
                                mm = nc.tensor.matmul(
                                    ps[:],
                                    w1_lhsT(f, h),
                                    xb[:, h, :],
                                    start=(h == 0),
                                    stop=(h == HO - 1),
                                )
                                if b == 0 and fl == 0:
                                    anchor[g * 8 + h] = mm
                            nc.scalar.activation(it[:, f, :], ps[:], GELU)

                for h in range(HO):
                    if b == NBLK - 1 and h == HO - 1:
                        # Final tile: run two column-half chains so only a
                        # half-width evict trails the last matmul, and evict
                        # the second half in quarters so its first store
                        # overlaps the second quarter's copy.
                        ob = opool.tile([P, T_BLK], bf16, tag="ob")
                        for half in range(2):
                            cols = slice(half * HB, (half + 1) * HB)
                            ps = psp.tile([P, T_BLK], f32, tag="pst")
                            for f in range(FO):
                                nc.tensor.matmul(
                                    ps[:, 0:HB],
                                    w2_sb[:, h, f * P:(f + 1) * P],
                                    it[:, f, cols],
                                    start=(f == 0),
                                    stop=(f == FO - 1),
                                )
                            if half == 0:
                                nc.vector.tensor_copy(ob[:, cols], ps[:, 0:HB])
                                nc.sync.dma_start(out4[b, h, :, cols], ob[:, cols])
                            else:
                                QB = HB // 2
                                for q in range(2):
                                    qc = slice(half * HB + q * QB,
                                               half * HB + (q + 1) * QB)
                                    nc.vector.tensor_copy(
                                        ob[:, qc], ps[:, q * QB:(q + 1) * QB])
                                    nc.sync.dma_start(out4[b, h, :, qc], ob[:, qc])
                        continue
                    ps = psp.tile([P, T_BLK], f32, tag="pst")
                    for f in range(FO):
                        nc.tensor.matmul(
                            ps[:],
                            w2_sb[:, h, f * P:(f + 1) * P],
                            it[:, f, :],
                            start=(f == 0),
                            stop=(f == FO - 1),
                        )
                    # Evict in two halves so the DMA store of the first half
                    # overlaps the copy of the second.
                    ob = opool.tile([P, T_BLK], bf16, tag="ob")
                    nc.vector.tensor_copy(ob[:, :HB], ps[:, :HB])
                    nc.sync.dma_start(out4[b, h, :, :HB], ob[:, :HB])
                    nc.vector.tensor_copy(ob[:, HB:], ps[:, HB:])
                    nc.sync.dma_start(out4[b, h, :, HB:], ob[:, HB:])

            # Stage the weight stream behind compute progress so the bulk of
            # the 16MB of weights never contends with the startup critical
            # path.  w1 chunk c is gated ~1.5 groups ahead of its first
            # consumer (the SP queue needs the slack: its copy backlog runs
            # at ~165 B/ns); w2 chunk c is gated on block 0's group-2
            # progress (w2 is first read ~55us in).
            for c in range(HO, NG * HO):
                add_dep_helper(
                    w1_dmas[c].ins, anchor[max(0, c - 12)].ins,
                    sync=True, reason="stage w1 load behind compute",
                )
            for c in range(NW2):
                add_dep_helper(
                    w2_dmas[c].ins, anchor[16 + c].ins,
                    sync=True, reason="stage w2 load behind compute",
                )
    nc.compile()
    return nc


def _get_nc():
    global _nc_cache
    if _nc_cache is None:
        _nc_cache = _build_nc()
    return _nc_cache


def kernel(x, w1, w2, rows_per_expert):
    global LAST_RESULTS
    from concourse.bass_utils import run_bass_kernel_spmd

    x = np.asarray(x)
    w1 = np.asarray(w1)
    w2 = np.asarray(w2)
    rpe = int(rows_per_expert)
    assert x.shape == (E * rpe, H) and rpe == T_PER_E
    assert w1.shape == (E, H, F) and w2.shape == (E, F, H)

    bf16 = ml_dtypes.bfloat16
    in_maps = []
    for e in range(E):
        xe = x[e * rpe:(e + 1) * rpe].astype(bf16)      # [T, H]
        # [b*T_BLK+ti, ho*128+p] -> [b, p, ho, ti]
        xpm = np.ascontiguousarray(
            xe.reshape(NBLK, T_BLK, HO, P).transpose(0, 3, 2, 1)
        )
        # w1[h*128+p, g*1024+q] -> chunk c=g*8+h: [c, p, q]
        w1m = np.ascontiguousarray(
            w1[e].astype(bf16).reshape(HO, P, NG, 1024).transpose(2, 0, 1, 3)
        ).reshape(NG * HO, P, 1024)
        # w2[fo*128+p, h*128+hi] -> [h, p, fo*128+hi]
        w2m = np.ascontiguousarray(
            w2[e].astype(bf16).reshape(FO, P, HO, P).transpose(2, 1, 0, 3)
        ).reshape(HO, P, F)
        in_maps.append({"xp": xpm, "w1p": w1m, "w2p": w2m})

    res = run_bass_kernel_spmd(_get_nc(), in_maps, list(range(E)), trace=TRACE)
    LAST_RESULTS = res

    out = np.empty((E * rpe, H), dtype=np.float32)
    for e in range(E):
        # [b, h, p, ti] -> [b*T_BLK+ti, h*128+p]
        o4 = res.results[e]["out4"].astype(np.float32)
        out[e * rpe:(e + 1) * rpe] = o4.transpose(0, 3, 1, 2).reshape(rpe, H)
    return out
